# revision 5
# baseline (speedup 1.0000x reference)
"""Multi-step LIF neuron (T=4) on 8 Trainium2 NeuronCores via Bass/Tile.

Reference recurrence (per element, v0 = 0):
    v_c  = v + (x - v) * 0.5        # tau = 2, v_reset = 0  (exact op order!)
    s    = (v_c >= 1.0)             # spike (forward value of the STE)
    v'   = 0 if s else v_c          # hard reset, detach
Output is s as float32 (0.0 / 1.0), shape [4, 128, 262144].

Sharding: pure data parallel over the batch dim. B=128 = 8 cores x 16 rows.
Each core sees x_shard [4, 128, 32768] (its 16 B-rows flattened to
128 SBUF partitions x 32768 free) and writes the same-shape spike shard.
The T recurrence is carried per element in SBUF; no cross-core traffic.
"""

import numpy as np

import concourse.bass as bass
import concourse.mybir as mybir
import concourse.tile as tile
from concourse import bacc
from concourse.bass_utils import run_bass_kernel_spmd

F32 = mybir.dt.float32

T = 4
B = 128
N = 262144
N_CORES = 8
ROWS_PER_CORE = B // N_CORES              # 16
FREE = ROWS_PER_CORE * N // 128           # 32768 free elems per partition
P = 128
TILE_F = 2048                             # free-dim tile: 1 MiB per DMA

_cache = {}


def _build_nc():
    nc = bacc.Bacc("TRN2", target_bir_lowering=False)
    x_d = nc.declare_dram_parameter("x", [T, P, FREE], F32, isOutput=False)
    s_d = nc.declare_dram_parameter("s", [T, P, FREE], F32, isOutput=True)

    with tile.TileContext(nc) as tc:
        with tc.tile_pool(name="io", bufs=2) as io, tc.tile_pool(name="work", bufs=1) as work:
            for j in range(FREE // TILE_F):
                js = bass.ts(j, TILE_F)
                xt = []
                for t in range(T):
                    xtile = io.tile([P, TILE_F], F32, tag=f"x{t}")
                    nc.sync.dma_start(out=xtile[:], in_=x_d[t, :, js])
                    xt.append(xtile)
                v = work.tile([P, TILE_F], F32, tag="v")
                vc = work.tile([P, TILE_F], F32, tag="vc")
                m = work.tile([P, TILE_F], F32, tag="m")
                for t in range(T):
                    st = io.tile([P, TILE_F], F32, tag=f"s{t}")
                    if t == 0:
                        # v0 = 0  ->  v_c = x * 0.5 (bit-identical to 0 + (x-0)*0.5)
                        nc.vector.tensor_scalar_mul(vc[:], xt[t][:], 0.5)
                    else:
                        d = work.tile([P, TILE_F], F32, tag="d")
                        nc.vector.tensor_tensor(d[:], xt[t][:], v[:], mybir.AluOpType.subtract)
                        nc.vector.tensor_scalar_mul(d[:], d[:], 0.5)
                        nc.vector.tensor_tensor(vc[:], v[:], d[:], mybir.AluOpType.add)
                    nc.vector.tensor_scalar(st[:], vc[:], 1.0, None, mybir.AluOpType.is_ge)
                    if t < T - 1:
                        nc.vector.tensor_scalar(m[:], vc[:], 1.0, None, mybir.AluOpType.is_lt)
                        nc.vector.tensor_tensor(v[:], vc[:], m[:], mybir.AluOpType.mult)
                    nc.sync.dma_start(out=s_d[t, :, js], in_=st[:])

    nc.compile()
    return nc


def _get_nc():
    if "nc" not in _cache:
        _cache["nc"] = _build_nc()
    return _cache["nc"]


def _shard(x_seq: np.ndarray) -> list[dict[str, np.ndarray]]:
    in_maps = []
    for c in range(N_CORES):
        xs = np.ascontiguousarray(
            x_seq[:, c * ROWS_PER_CORE:(c + 1) * ROWS_PER_CORE, :]
        ).reshape(T, P, FREE)
        in_maps.append({"x": xs})
    return in_maps


def _unshard(results: list[dict[str, np.ndarray]]) -> np.ndarray:
    parts = [r["s"].reshape(T, ROWS_PER_CORE, N) for r in results]
    return np.concatenate(parts, axis=1)


def kernel(x_seq: np.ndarray) -> np.ndarray:
    x_seq = np.asarray(x_seq, dtype=np.float32)
    assert x_seq.shape == (T, B, N), x_seq.shape
    nc = _get_nc()
    res = run_bass_kernel_spmd(nc, _shard(x_seq), core_ids=list(range(N_CORES)))
    return _unshard(res.results)


# ---------------------------------------------------------------- benchmarking
def _make_exec(nc):
    """Build the sharded jitted executable once (mirrors run_bass_via_pjrt)."""
    import jax
    from jax.sharding import Mesh, PartitionSpec
    from jax.experimental.shard_map import shard_map
    from concourse import bass2jax

    bass2jax.install_neuronx_cc_hook()

    partition_name = nc.partition_id_tensor.name if nc.partition_id_tensor else None
    in_names, out_names, out_avals, zero_outs = [], [], [], []
    for alloc in nc.m.functions[0].allocations:
        if not isinstance(alloc, mybir.MemoryLocationSet):
            continue
        name = alloc.memorylocations[0].name
        if alloc.kind == "ExternalInput":
            if name != partition_name:
                in_names.append(name)
        elif alloc.kind == "ExternalOutput":
            shape = tuple(alloc.tensor_shape)
            dtype = mybir.dt.np(alloc.dtype)
            out_names.append(name)
            out_avals.append(jax.core.ShapedArray(shape, dtype))
            zero_outs.append(np.zeros(shape, dtype))
    n_params = len(in_names)
    n_outs = len(out_avals)
    all_in_names = in_names + out_names
    if partition_name is not None:
        all_in_names.append(partition_name)
    donate = tuple(range(n_params, n_params + n_outs))

    def _body(*args):
        operands = list(args)
        if partition_name is not None:
            operands.append(bass2jax.partition_id_tensor())
        outs = bass2jax._bass_exec_p.bind(
            *operands,
            out_avals=tuple(out_avals),
            in_names=tuple(all_in_names),
            out_names=tuple(out_names),
            lowering_input_output_aliases=(),
            sim_require_finite=True,
            sim_require_nnan=True,
            nc=nc,
        )
        return tuple(outs)

    devices = jax.devices()[:N_CORES]
    mesh = Mesh(np.asarray(devices), ("core",))
    in_specs = (PartitionSpec("core"),) * (n_params + n_outs)
    out_specs = (PartitionSpec("core"),) * n_outs
    f = jax.jit(
        shard_map(_body, mesh=mesh, in_specs=in_specs, out_specs=out_specs,
                  check_rep=False),
        donate_argnums=donate, keep_unused=True,
    )
    return f, mesh, in_names, out_names, zero_outs


def bench(x_seq: np.ndarray, repeats: int = 10):
    """Time pure device executions (warm executable, inputs resident on
    device, output buffers donated ping-pong). Returns min ns per exec."""
    import time
    import jax
    from jax.sharding import NamedSharding, PartitionSpec

    x_seq = np.asarray(x_seq, dtype=np.float32)
    nc = _get_nc()
    f, mesh, in_names, out_names, zero_outs = _make_exec(nc)

    in_maps = _shard(x_seq)
    concat_in = [
        np.concatenate([m[name] for m in in_maps], axis=0) for name in in_names
    ]
    sh = NamedSharding(mesh, PartitionSpec("core"))
    xc = [jax.device_put(a, sh) for a in concat_in]
    zc = [
        jax.device_put(np.zeros((N_CORES * z.shape[0], *z.shape[1:]), z.dtype), sh)
        for z in zero_outs
    ]
    outs = f(*xc, *zc)  # warm-up (compiles)
    jax.block_until_ready(outs)
    times = []
    for _ in range(repeats):
        t0 = time.perf_counter()
        outs = f(*xc, *outs)
        jax.block_until_ready(outs)
        times.append(time.perf_counter() - t0)
    times.sort()
    print("bench times (s):", [f"{t:.6f}" for t in times])
    return times[0] * 1e9


# revision 11
# speedup vs baseline: 43.2013x; 43.2013x over previous
"""Multi-step LIF neuron (T=4) on 8 Trainium2 NeuronCores via Bass/Tile.

Reference recurrence (per element, v0 = 0):
    v_c  = v + (x - v) * 0.5        # tau = 2, v_reset = 0  (exact op order!)
    s    = (v_c >= 1.0)             # spike (forward value of the STE)
    v'   = 0 if s else v_c          # hard reset, detach
Output is s as float32 (0.0 / 1.0), shape [4, 128, 262144].

Sharding: pure data parallel over the batch dim. B=128 = 8 cores x 16 rows.
Each core sees x_shard [4, 128, 32768] (its 16 B-rows flattened to
128 SBUF partitions x 32768 free) and writes the same-shape spike shard.
The T recurrence is carried per element in SBUF; no cross-core traffic.
"""

import numpy as np

import concourse.bass as bass
import concourse.mybir as mybir
import concourse.tile as tile
from concourse import bacc
from concourse.bass_utils import run_bass_kernel_spmd

F32 = mybir.dt.float32

T = 4
B = 128
N = 262144
N_CORES = 8
ROWS_PER_CORE = B // N_CORES              # 16
FREE = ROWS_PER_CORE * N // 128           # 32768 free elems per partition
P = 128
TILE_F = 2048                             # free-dim tile: 1 MiB per DMA

_cache = {}


def _build_nc(rep: int = 1):
    nc = bacc.Bacc("TRN2", target_bir_lowering=False)
    x_d = nc.declare_dram_parameter("x", [T, P, FREE], F32, isOutput=False)
    s_d = nc.declare_dram_parameter("s", [T, P, FREE], F32, isOutput=True)
    scratch = [
        nc.dram_tensor(f"s_scratch{r}", [T, P, FREE], F32) for r in range(rep - 1)
    ]

    with tile.TileContext(nc) as tc:
        with tc.tile_pool(name="io", bufs=2) as io, tc.tile_pool(name="work", bufs=1) as work:
            for r in range(rep):
                out_d = s_d if r == 0 else scratch[r - 1]
                for j in range(FREE // TILE_F):
                    js = bass.ts(j, TILE_F)
                    xt = []
                    for t in range(T):
                        xtile = io.tile([P, TILE_F], F32, tag=f"x{t}")
                        nc.sync.dma_start(out=xtile[:], in_=x_d[t, :, js])
                        xt.append(xtile)
                    v = work.tile([P, TILE_F], F32, tag="v")
                    vc = work.tile([P, TILE_F], F32, tag="vc")
                    m = work.tile([P, TILE_F], F32, tag="m")
                    for t in range(T):
                        st = io.tile([P, TILE_F], F32, tag=f"s{t}")
                        if t == 0:
                            # v0 = 0  ->  v_c = x * 0.5 (bit-identical to 0 + (x-0)*0.5)
                            nc.vector.tensor_scalar_mul(vc[:], xt[t][:], 0.5)
                        else:
                            d = work.tile([P, TILE_F], F32, tag="d")
                            nc.vector.tensor_tensor(d[:], xt[t][:], v[:], mybir.AluOpType.subtract)
                            nc.vector.tensor_scalar_mul(d[:], d[:], 0.5)
                            nc.vector.tensor_tensor(vc[:], v[:], d[:], mybir.AluOpType.add)
                        nc.vector.tensor_scalar(st[:], vc[:], 1.0, None, mybir.AluOpType.is_ge)
                        if t < T - 1:
                            nc.vector.tensor_scalar(m[:], vc[:], 1.0, None, mybir.AluOpType.is_lt)
                            nc.vector.tensor_tensor(v[:], vc[:], m[:], mybir.AluOpType.mult)
                        nc.sync.dma_start(out=out_d[t, :, js], in_=st[:])

    nc.compile()
    return nc


def _get_nc(rep: int = 1):
    key = f"nc{rep}"
    if key not in _cache:
        _cache[key] = _build_nc(rep)
    return _cache[key]


def _shard(x_seq: np.ndarray) -> list[dict[str, np.ndarray]]:
    in_maps = []
    for c in range(N_CORES):
        xs = np.ascontiguousarray(
            x_seq[:, c * ROWS_PER_CORE:(c + 1) * ROWS_PER_CORE, :]
        ).reshape(T, P, FREE)
        in_maps.append({"x": xs})
    return in_maps


def _unshard(results: list[dict[str, np.ndarray]]) -> np.ndarray:
    parts = [r["s"].reshape(T, ROWS_PER_CORE, N) for r in results]
    return np.concatenate(parts, axis=1)


def kernel(x_seq: np.ndarray) -> np.ndarray:
    x_seq = np.asarray(x_seq, dtype=np.float32)
    assert x_seq.shape == (T, B, N), x_seq.shape
    nc = _get_nc()
    res = run_bass_kernel_spmd(nc, _shard(x_seq), core_ids=list(range(N_CORES)))
    return _unshard(res.results)


# ---------------------------------------------------------------- benchmarking
def _make_exec(nc):
    """Build the sharded jitted executable once (mirrors run_bass_via_pjrt)."""
    import jax
    from jax.sharding import Mesh, PartitionSpec
    from jax.experimental.shard_map import shard_map
    from concourse import bass2jax

    bass2jax.install_neuronx_cc_hook()

    partition_name = nc.partition_id_tensor.name if nc.partition_id_tensor else None
    in_names, out_names, out_avals, zero_outs = [], [], [], []
    for alloc in nc.m.functions[0].allocations:
        if not isinstance(alloc, mybir.MemoryLocationSet):
            continue
        name = alloc.memorylocations[0].name
        if alloc.kind == "ExternalInput":
            if name != partition_name:
                in_names.append(name)
        elif alloc.kind == "ExternalOutput":
            shape = tuple(alloc.tensor_shape)
            dtype = mybir.dt.np(alloc.dtype)
            out_names.append(name)
            out_avals.append(jax.core.ShapedArray(shape, dtype))
            zero_outs.append(np.zeros(shape, dtype))
    n_params = len(in_names)
    n_outs = len(out_avals)
    all_in_names = in_names + out_names
    if partition_name is not None:
        all_in_names.append(partition_name)
    donate = tuple(range(n_params, n_params + n_outs))

    def _body(*args):
        operands = list(args)
        if partition_name is not None:
            operands.append(bass2jax.partition_id_tensor())
        outs = bass2jax._bass_exec_p.bind(
            *operands,
            out_avals=tuple(out_avals),
            in_names=tuple(all_in_names),
            out_names=tuple(out_names),
            lowering_input_output_aliases=(),
            sim_require_finite=True,
            sim_require_nnan=True,
            nc=nc,
        )
        return tuple(outs)

    devices = jax.devices()[:N_CORES]
    mesh = Mesh(np.asarray(devices), ("core",))
    in_specs = (PartitionSpec("core"),) * (n_params + n_outs)
    out_specs = (PartitionSpec("core"),) * n_outs
    f = jax.jit(
        shard_map(_body, mesh=mesh, in_specs=in_specs, out_specs=out_specs,
                  check_rep=False),
        donate_argnums=donate, keep_unused=True,
    )
    return f, mesh, in_names, out_names, zero_outs


def _time_rep(x_seq, rep, repeats):
    import time
    import jax
    from jax.sharding import NamedSharding, PartitionSpec

    nc = _get_nc(rep)
    f, mesh, in_names, out_names, zero_outs = _make_exec(nc)

    in_maps = _shard(x_seq)
    concat_in = [
        np.concatenate([m[name] for m in in_maps], axis=0) for name in in_names
    ]
    sh = NamedSharding(mesh, PartitionSpec("core"))
    xc = [jax.device_put(a, sh) for a in concat_in]
    zc = [
        jax.device_put(np.zeros((N_CORES * z.shape[0], *z.shape[1:]), z.dtype), sh)
        for z in zero_outs
    ]
    outs = f(*xc, *zc)  # warm-up (compiles)
    jax.block_until_ready(outs)
    times = []
    for _ in range(repeats):
        t0 = time.perf_counter()
        outs = f(*xc, *outs)
        jax.block_until_ready(outs)
        times.append(time.perf_counter() - t0)
    times.sort()
    return times


def bench(x_seq: np.ndarray, repeats: int = 10, rep: int = 5):
    """Estimate per-execution device time: marginal cost of extra in-kernel
    repetitions of the full pipeline (cancels RPC/dispatch overhead)."""
    x_seq = np.asarray(x_seq, dtype=np.float32)
    t1 = _time_rep(x_seq, 1, repeats)
    tk = _time_rep(x_seq, rep, repeats)
    print(f"rep=1 times: {[f'{t:.6f}' for t in t1]}")
    print(f"rep={rep} times: {[f'{t:.6f}' for t in tk]}")
    marginal = (tk[0] - t1[0]) / (rep - 1)
    print(f"rep=1 min: {t1[0]*1e3:.3f} ms; rep={rep} min: {tk[0]*1e3:.3f} ms; "
          f"marginal per exec: {marginal*1e3:.3f} ms")
    return marginal * 1e9
